# revision 1
# baseline (speedup 1.0000x reference)
"""Bloom attention (separated QKV) — 8-core TRN2 Bass kernel.

Distribution: tensor-parallel over heads (2 heads/core). Each core:
  1. QKV projections for its 256-row slice of Wq/Wk/Wv (q^T,k^T in [d,s]
     layout, v in [s,d] layout, all bf16 in SBUF, fp32 accumulate).
  2. Attention with transposed scores St[k,q] = k @ q^T computed in
     qq=1024 groups, exp via ScalarE (alibi as per-partition bias),
     softmax denominator via ones-matmul, ctx^T = v^T @ P in PSUM,
     normalized by broadcast 1/den.
  3. Chunked AllGather (4 chunks along the sequence) of ctx^T slices
     (bf16), overlapped with the remaining attention blocks.
  4. Output projection for its 256-column slice of Wd + bias + residual,
     per gathered chunk.
Host side: transpose/slice/cast weights + hs (layout prep only),
concatenate the 8 output column-slices.
"""
import numpy as np
import ml_dtypes

import concourse.bass as bass
import concourse.bacc as bacc
import concourse.mybir as mybir
import concourse.tile as tile
import concourse.bass_utils as bass_utils

BF16 = ml_dtypes.bfloat16
N_CORES = 8
B, S, H = 2, 2048, 2048
NH, HD = 16, 128
HPC = NH // N_CORES          # heads per core
CI = HPC * HD                # per-core slice of H (256)
BS = B * S                   # 4096
INV_NORM = 1.0 / float(np.sqrt(HD))

JT = H // 128                # 16 contraction tiles for projections
SS_CHUNK = 512               # seq chunk for projections
N_CHUNKS = BS // SS_CHUNK    # 8
KT = S // 128                # 16 key tiles per batch
IT = H // 128                # 16 contraction tiles for dense
QBLK = 1024                  # attention/AG/dense block along seq
N_BLOCKS = BS // QBLK        # 4

F32 = mybir.dt.float32
BF = mybir.dt.bfloat16

DEBUG_OUTPUTS = False


def _build():
    nc = bacc.Bacc("TRN2", target_bir_lowering=False, debug=False,
                   num_devices=N_CORES)

    # hsT/weights are host-packed to the exact SBUF layouts so every DMA
    # is contiguous per partition (strided weight loads measured ~5x slower)
    hsT = nc.dram_tensor("hsT", [128, N_CHUNKS, JT, SS_CHUNK], BF,
                         kind="ExternalInput").ap()
    wqT = nc.dram_tensor("wqT", [128, JT, CI], BF, kind="ExternalInput").ap()
    wkT = nc.dram_tensor("wkT", [128, JT, CI], BF, kind="ExternalInput").ap()
    wvT = nc.dram_tensor("wvT", [128, JT, CI], BF, kind="ExternalInput").ap()
    wdT = nc.dram_tensor("wdT", [128, IT, CI], BF, kind="ExternalInput").ap()
    bq = nc.dram_tensor("bq", [CI, 1], F32, kind="ExternalInput").ap()
    bk = nc.dram_tensor("bk", [CI, 1], F32, kind="ExternalInput").ap()
    bv = nc.dram_tensor("bv", [1, CI], BF, kind="ExternalInput").ap()
    bd_f32 = nc.dram_tensor("bd", [CI, 1], F32, kind="ExternalInput").ap()
    alibi = nc.dram_tensor("alibi", [B * HPC, S], F32, kind="ExternalInput").ap()
    residT = nc.dram_tensor("residT", [CI, BS], F32, kind="ExternalInput").ap()
    outT = nc.dram_tensor("outT", [CI, BS], F32, kind="ExternalOutput").ap()

    bounce = nc.dram_tensor("bounce", [N_BLOCKS, HPC, 128, QBLK], BF,
                            kind="Internal").ap()
    # per-(block, hi) AllGather output: rows = core*128 + d
    gath = nc.dram_tensor("gath", [HPC, N_BLOCKS, N_CORES * 128, QBLK], BF,
                          kind="Internal", addr_space="Shared").ap()
    if DEBUG_OUTPUTS:
        qT_dbg = nc.dram_tensor("qT_dbg", [128, HPC * BS], BF,
                                kind="ExternalOutput").ap()
        kT_dbg = nc.dram_tensor("kT_dbg", [128, HPC * BS], BF,
                                kind="ExternalOutput").ap()
        v_dbg = nc.dram_tensor("v_dbg", [128, (BS // 128) * CI], BF,
                               kind="ExternalOutput").ap()
        ctxT_dbg = nc.dram_tensor("ctxT_dbg", [H, BS], BF,
                                  kind="ExternalOutput").ap()

    with tile.TileContext(nc) as tc:
        with (
            tc.tile_pool(name="const", bufs=1) as constp,
            tc.tile_pool(name="qkv", bufs=1) as qkvp,
            tc.tile_pool(name="ctile", bufs=18) as ctp,
        ):
            # ---- phase 0: constants (phase-1 critical ones first) ----
            wq_sb = constp.tile([128, JT, CI], BF)
            wk_sb = constp.tile([128, JT, CI], BF)
            wv_sb = constp.tile([128, JT, CI], BF)
            # wq on the fast gpsimd queue first; wk/wv are emitted inside
            # the chunk loop right after the first hs chunk so the queue
            # order is wq, hs0, wk, wv, hs1, ...
            nc.gpsimd.dma_start(wq_sb[:], wqT[:])
            bq_sb = constp.tile([128, HPC], F32)
            bk_sb = constp.tile([128, HPC], F32)
            for b_sb, b_dr in ((bq_sb, bq), (bk_sb, bk)):
                for hi in range(HPC):
                    nc.scalar.dma_start(b_sb[:, hi:hi + 1],
                                        b_dr[hi * 128:(hi + 1) * 128, :])
            bv_sb = constp.tile([1, CI], BF)
            nc.scalar.dma_start(bv_sb[:], bv[:])
            alibi_sb = constp.tile([128, B * HPC, KT], F32)
            nc.scalar.dma_start(
                alibi_sb[:], alibi.rearrange("r (kt p) -> p r kt", p=128))
            ones_col_f32 = constp.tile([128, 1], F32)  # den lhsT (K=128, M=1)
            ones_row_bf = constp.tile([1, 128], BF)    # bias lhsT (K=1, M=128)
            ones_row_f32 = constp.tile([1, 128], F32)  # bcast lhsT (K=1, M=128)
            nc.vector.memset(ones_col_f32[:], 1.0)
            nc.vector.memset(ones_row_bf[:], 1.0)
            nc.vector.memset(ones_row_f32[:], 1.0)

            # persistent per-core activations
            qT_sb = qkvp.tile([128, HPC, BS], BF)      # [d, hi, ss]
            kT_sb = qkvp.tile([128, HPC, BS], BF)
            v_sb = qkvp.tile([128, BS // 128, CI], BF)  # [ss%128, ss//128, i]

            # ---- phase 1: QKV projections ----
            with (
                tc.tile_pool(name="hsb", bufs=3) as hsp,
                tc.tile_pool(name="p1psum", bufs=4,
                             space=bass.MemorySpace.PSUM) as p1p,
            ):
                for ch in range(N_CHUNKS):
                    s0 = ch * SS_CHUNK
                    hsb = hsp.tile([128, JT, SS_CHUNK], BF, name="hsb")
                    nc.gpsimd.dma_start(hsb[:], hsT[:, ch])
                    if ch == 0:
                        nc.gpsimd.dma_start(wk_sb[:], wkT[:])
                        nc.gpsimd.dma_start(wv_sb[:], wvT[:])
                    for w_sb, b_col, o_sb, scale in (
                        (wq_sb, bq_sb, qT_sb, INV_NORM),
                        (wk_sb, bk_sb, kT_sb, 1.0),
                    ):
                        for hi in range(HPC):
                            ps = p1p.tile([128, SS_CHUNK], F32, name="ps_qk")
                            for jt in range(JT):
                                nc.tensor.matmul(
                                    ps[:],
                                    w_sb[:, jt, hi * 128:(hi + 1) * 128],
                                    hsb[:, jt, :],
                                    start=(jt == 0), stop=(jt == JT - 1))
                            nc.scalar.activation(
                                o_sb[:, hi, s0:s0 + SS_CHUNK], ps[:],
                                mybir.ActivationFunctionType.Identity,
                                bias=b_col[:, hi:hi + 1], scale=scale)
                    for st in range(SS_CHUNK // 128):
                        ps = p1p.tile([128, CI], F32, name="ps_v")
                        nc.tensor.matmul(ps[:], ones_row_bf[:], bv_sb[:],
                                         start=True, stop=False)
                        for jt in range(JT):
                            nc.tensor.matmul(
                                ps[:],
                                hsb[:, jt, st * 128:(st + 1) * 128],
                                wv_sb[:, jt, :],
                                start=False, stop=(jt == JT - 1))
                        nc.scalar.copy(v_sb[:, ch * 4 + st, :], ps[:])

            # late consts (dense phase only) — declared after phase 1 so
            # their DMAs don't delay the first projections
            wd_sb = constp.tile([128, IT, CI], BF)
            nc.sync.dma_start(wd_sb[:], wdT[:])
            bd_col = constp.tile([128, HPC], F32)
            for ci in range(HPC):
                nc.sync.dma_start(bd_col[:, ci:ci + 1],
                                  bd_f32[ci * 128:(ci + 1) * 128, :])

            # ---- phase 2+3: attention blocks + chunked AllGather ----
            with (
                tc.tile_pool(name="stp", bufs=3,
                             space=bass.MemorySpace.PSUM) as stp,
                tc.tile_pool(name="ptp", bufs=12) as ptp,
                tc.tile_pool(name="accp", bufs=1,
                             space=bass.MemorySpace.PSUM) as accp,
                tc.tile_pool(name="normp", bufs=2) as normp,
            ):
                LAG = 6
                pending_tail = [None]
                ctiles = {}
                it_order = [*range(0, IT, 2), *range(1, IT, 2)]

                def prefetch_ctiles(blk):
                    tiles = []
                    for it in it_order:
                        ctile = ctp.tile([128, QBLK], BF, name="ctile")
                        nc.gpsimd.dma_start(
                            ctile[:],
                            gath[it % HPC, blk,
                                 (it // HPC) * 128:(it // HPC + 1) * 128, :])
                        tiles.append(ctile)
                    ctiles[blk] = tiles

                def flush_tail():
                    if pending_tail[0] is not None:
                        pending_tail[0]()
                        pending_tail[0] = None

                for blk in range(N_BLOCKS):
                    b, qh = divmod(blk, N_BLOCKS // B)
                    q0 = b * S + qh * QBLK
                    for hi in range(HPC):
                        bh = b * HPC + hi
                        ctx_ps = accp.tile([128, QBLK], F32, name="ctx_ps")
                        acc_sb = normp.tile([128, QBLK], F32, name="acc_sb")
                        pts = []

                        def consume(kt, ctx_ps=ctx_ps, acc_sb=acc_sb,
                                    pts=pts, b=b, hi=hi):
                            pt = pts[kt]
                            for half in range(2):
                                hs_ = slice(half * SS_CHUNK,
                                            (half + 1) * SS_CHUNK)
                                nc.tensor.matmul(
                                    ctx_ps[:, hs_],
                                    v_sb[:, (b * S) // 128 + kt,
                                         hi * 128:(hi + 1) * 128],
                                    pt[:, half, :],
                                    start=(kt == 0), stop=(kt == KT - 1))
                            # denominator partial sums on DVE (off PE):
                            # bf16 pair-sum (2x DVE rate), f32 chain
                            if kt % 2 == 1:
                                pa = pts[kt - 1][:].rearrange(
                                    "p a b -> p (a b)")
                                pb = pt[:].rearrange("p a b -> p (a b)")
                                psum2 = normp.tile([128, QBLK], BF,
                                                   name="psum2")
                                nc.vector.tensor_add(psum2[:], pa, pb)
                                if kt == 1:
                                    nc.vector.tensor_copy(acc_sb[:],
                                                          psum2[:])
                                else:
                                    nc.vector.tensor_add(acc_sb[:],
                                                         acc_sb[:],
                                                         psum2[:])

                        for kt in range(KT):
                            k0 = b * S + kt * 128
                            st_ps = stp.tile([128, 2, SS_CHUNK], F32,
                                             name="st_ps")
                            for half in range(2):
                                nc.tensor.matmul(
                                    st_ps[:, half, :],
                                    kT_sb[:, hi, k0:k0 + 128],
                                    qT_sb[:, hi,
                                          q0 + half * SS_CHUNK:
                                          q0 + (half + 1) * SS_CHUNK],
                                    start=True, stop=True)
                            pt = ptp.tile([128, 2, SS_CHUNK], BF, name="pt")
                            # q pre-scaled by INV_NORM in phase 1; alibi is
                            # a per-partition (key-position) bias
                            nc.scalar.activation(
                                pt[:], st_ps[:],
                                mybir.ActivationFunctionType.Exp,
                                bias=alibi_sb[:, bh, kt:kt + 1])
                            pts.append(pt)
                            # previous group's normalize tail slots in
                            # behind our first few St/exp emissions
                            if kt == 2:
                                flush_tail()
                            if kt >= LAG:
                                consume(kt - LAG)
                        for kt in range(KT - LAG, KT):
                            consume(kt)
                        # cross-partition reduce of acc -> den (borrows an
                        # stp slot; acc chain finishes under the last ctx
                        # matmuls)
                        den_ps = stp.tile([128, 2, SS_CHUNK], F32,
                                          name="st_ps")
                        for half in range(2):
                            nc.tensor.matmul(
                                den_ps[:1, half, :], ones_col_f32[:],
                                acc_sb[:, half * SS_CHUNK:
                                       (half + 1) * SS_CHUNK],
                                start=True, stop=True)
                        den_sb = normp.tile([1, QBLK], F32, name="den_sb")
                        # ACT is idle at the group boundary; DVE is draining
                        # the pair-sum chain — use ACT for this copy
                        nc.scalar.copy(
                            den_sb[:],
                            den_ps[:1, :, :].rearrange("p a b -> p (a b)"))

                        def tail(ctx_ps=ctx_ps, den_sb=den_sb, blk=blk,
                                 hi=hi):
                            denb_ps = stp.tile([128, 2, SS_CHUNK], F32,
                                               name="st_ps")
                            for half in range(2):
                                nc.tensor.matmul(
                                    denb_ps[:, half, :], ones_row_f32[:],
                                    den_sb[:, half * SS_CHUNK:
                                           (half + 1) * SS_CHUNK],
                                    start=True, stop=True)
                            denb_sb = normp.tile([128, QBLK], F32,
                                                 name="denb_sb")
                            nc.vector.reciprocal_approx_fast(
                                denb_sb[:],
                                denb_ps[:].rearrange("p a b -> p (a b)"))
                            ctxn_sb = normp.tile([128, QBLK], BF,
                                                 name="ctxn_sb")
                            nc.vector.tensor_mul(ctxn_sb[:], ctx_ps[:],
                                                 denb_sb[:])
                            nc.sync.dma_start(bounce[blk, hi], ctxn_sb[:])
                            nc.gpsimd.collective_compute(
                                "AllGather", mybir.AluOpType.bypass,
                                replica_groups=[list(range(N_CORES))],
                                ins=[bounce[blk, hi]],
                                outs=[gath[hi, blk]])
                            # block-0 ctile prefetch slots in on the gpsimd
                            # queue once its gathers are long done
                            if (blk, hi) == (1, 0):
                                prefetch_ctiles(0)

                        pending_tail[0] = tail
                flush_tail()
                for blk in range(1, N_BLOCKS):
                    prefetch_ctiles(blk)

            if DEBUG_OUTPUTS:
                nc.sync.dma_start(qT_dbg[:],
                                  qT_sb[:].rearrange("p a b -> p (a b)"))
                nc.sync.dma_start(kT_dbg[:],
                                  kT_sb[:].rearrange("p a b -> p (a b)"))
                nc.sync.dma_start(v_dbg[:],
                                  v_sb[:].rearrange("p a b -> p (a b)"))
                dbg_r = ctxT_dbg.rearrange("(c x d) s -> c x d s", x=HPC,
                                           d=128)
                for blk in range(N_BLOCKS):
                    b, qh = divmod(blk, N_BLOCKS // B)
                    q0 = b * S + qh * QBLK
                    for hi in range(HPC):
                        nc.sync.dma_start(
                            dbg_r[:, hi, :, q0:q0 + QBLK],
                            gath[hi, blk].rearrange("(c d) s -> c d s",
                                                    d=128))

            # ---- phase 4: output projection (out^T form: Wd stationary,
            # LDWEIGHTS amortized over the moving ctx^T) + bias + residual
            with (
                tc.tile_pool(name="dpsum", bufs=8,
                             space=bass.MemorySpace.PSUM) as dpp,
                tc.tile_pool(name="outp", bufs=4) as outp,
            ):
                NSC = QBLK // SS_CHUNK      # 2 seq chunks per block
                for blk in range(N_BLOCKS):
                    b, qh = divmod(blk, N_BLOCKS // B)
                    q0 = b * S + qh * QBLK
                    dps = [dpp.tile([128, SS_CHUNK], F32, name="dps")
                           for _ in range(HPC * NSC)]
                    # hi=0 rows (even it) first: their AllGather chunk
                    # lands one attention group earlier than hi=1's
                    for j, it in enumerate(it_order):
                        ctile = ctiles[blk][j]
                        for ct in range(HPC):
                            for sc in range(NSC):
                                nc.tensor.matmul(
                                    dps[ct * NSC + sc][:],
                                    wd_sb[:, it, ct * 128:(ct + 1) * 128],
                                    ctile[:, sc * SS_CHUNK:
                                          (sc + 1) * SS_CHUNK],
                                    start=(j == 0), stop=(j == IT - 1))
                    for ct in range(HPC):
                        for sc in range(NSC):
                            c0 = ct * 128
                            s0_ = q0 + sc * SS_CHUNK
                            rtile = outp.tile([128, SS_CHUNK], F32,
                                              name="rtile")
                            nc.sync.dma_start(
                                rtile[:],
                                residT[c0:c0 + 128, s0_:s0_ + SS_CHUNK])
                            # bias is per-partition (output channel) here
                            osb = outp.tile([128, SS_CHUNK], F32,
                                            name="osb")
                            nc.scalar.activation(
                                osb[:], dps[ct * NSC + sc][:],
                                mybir.ActivationFunctionType.Identity,
                                bias=bd_col[:, ct:ct + 1])
                            osb2 = outp.tile([128, SS_CHUNK], F32,
                                             name="osb2")
                            nc.vector.tensor_add(osb2[:], osb[:], rtile[:])
                            nc.sync.dma_start(
                                outT[c0:c0 + 128, s0_:s0_ + SS_CHUNK],
                                osb2[:])

    nc.compile()
    return nc


_NC = None


def _get_nc():
    global _NC
    if _NC is None:
        _NC = _build()
    return _NC


def _pack_w(W, sl):
    # [H, CI] transposed slice -> SBUF layout [128, JT, CI], contiguous
    wT = np.asarray(W, np.float32)[sl].T            # [H, CI]
    return np.ascontiguousarray(
        wT.reshape(JT, 128, CI).transpose(1, 0, 2)).astype(BF16)


def _prep_in_maps(hidden_states, residual, alibi, Wq, bq, Wk, bk, Wv, bv,
                  Wd, bd):
    hs = np.ascontiguousarray(np.asarray(hidden_states, np.float32)
                              .reshape(BS, H))
    # SBUF chunk layout [128, ch, jt, s]: element = hs[ch*512+s, jt*128+p]
    hs_pack = np.ascontiguousarray(
        hs.reshape(N_CHUNKS, SS_CHUNK, JT, 128).transpose(3, 0, 2, 1)
    ).astype(BF16)
    resid = np.asarray(residual, np.float32).reshape(BS, H)
    alibi_r = np.asarray(alibi, np.float32).reshape(B, NH, S)
    in_maps = []
    for c in range(N_CORES):
        sl = slice(c * CI, (c + 1) * CI)
        # alibi rows ordered (b, hi) to match kernel indexing bh = b*HPC+hi
        al = np.ascontiguousarray(
            alibi_r[:, c * HPC:(c + 1) * HPC, :].reshape(B * HPC, S))
        in_maps.append({
            "hsT": hs_pack,
            "wqT": _pack_w(Wq, sl),
            "wkT": _pack_w(Wk, sl),
            "wvT": _pack_w(Wv, sl),
            "wdT": _pack_w(Wd, sl),
            "bq": np.asarray(bq, np.float32)[sl].reshape(CI, 1),
            "bk": np.asarray(bk, np.float32)[sl].reshape(CI, 1),
            "bv": np.asarray(bv, np.float32)[sl].reshape(1, CI).astype(BF16),
            "bd": np.asarray(bd, np.float32)[sl].reshape(CI, 1),
            "alibi": al,
            "residT": np.ascontiguousarray(resid[:, sl].T),
        })
    return in_maps


def run(trace=False, trace_cores=None, stitch_traces=False, **inputs):
    nc = _get_nc()
    in_maps = _prep_in_maps(**inputs)
    res = bass_utils.run_bass_kernel_spmd(
        nc, in_maps, core_ids=list(range(N_CORES)), trace=trace,
        trace_cores=trace_cores, stitch_traces=stitch_traces)
    full = np.empty((BS, H), np.float32)
    for c in range(N_CORES):
        full[:, c * CI:(c + 1) * CI] = res.results[c]["outT"].T
    return full.reshape(B, S, H), res


def kernel(**inputs):
    out, _ = run(trace=False, **inputs)
    return out



# revision 17
# speedup vs baseline: 1.4084x; 1.4084x over previous
"""Bloom attention (separated QKV) — 8-core TRN2 Bass kernel.

Distribution: tensor-parallel over heads (2 heads/core). Each core:
  1. QKV projections for its 256-row slice of Wq/Wk/Wv (q^T,k^T in [d,s]
     layout, v in [s,d] layout, all bf16 in SBUF, fp32 accumulate).
  2. Attention with transposed scores St[k,q] = k @ q^T computed in
     qq=1024 groups, exp via ScalarE (alibi as per-partition bias),
     softmax denominator via ones-matmul, ctx^T = v^T @ P in PSUM,
     normalized by broadcast 1/den.
  3. Chunked AllGather (4 chunks along the sequence) of ctx^T slices
     (bf16), overlapped with the remaining attention blocks.
  4. Output projection for its 256-column slice of Wd + bias + residual,
     per gathered chunk.
Host side: transpose/slice/cast weights + hs (layout prep only),
concatenate the 8 output column-slices.
"""
import numpy as np
import ml_dtypes

import concourse.bass as bass
import concourse.bacc as bacc
import concourse.mybir as mybir
import concourse.tile as tile
import concourse.bass_utils as bass_utils

BF16 = ml_dtypes.bfloat16
F8NP = ml_dtypes.float8_e4m3
N_CORES = 8
B, S, H = 2, 2048, 2048
NH, HD = 16, 128
HPC = NH // N_CORES          # heads per core
CI = HPC * HD                # per-core slice of H (256)
BS = B * S                   # 4096
INV_NORM = 1.0 / float(np.sqrt(HD))
WS = 64.0                    # fp8 weight pre-scale (host), undone in ACT
CS = 32.0                    # ctx pre-scale before fp8 cast, undone in ACT

JT = H // 128                # 16 contraction tiles for projections
JP = JT // 2                 # 8 DoubleRow k-tile pairs
SS_CHUNK = 512               # seq chunk for projections
N_CHUNKS = BS // SS_CHUNK    # 8
KT = S // 128                # 16 key tiles per batch
IT = H // 128                # 16 contraction tiles for dense
QBLK = 1024                  # attention/AG/dense block along seq
N_BLOCKS = BS // QBLK        # 4

F32 = mybir.dt.float32
BF = mybir.dt.bfloat16
FP8 = mybir.dt.float8e4
DR = mybir.MatmulPerfMode.DoubleRow

DEBUG_OUTPUTS = False


def _build():
    nc = bacc.Bacc("TRN2", target_bir_lowering=False, debug=False,
                   num_devices=N_CORES)

    # hsT/weights are host-packed to the exact SBUF layouts so every DMA
    # is contiguous per partition (strided weight loads measured ~5x slower)
    hsT = nc.dram_tensor("hsT", [128, N_CHUNKS, JT, SS_CHUNK], FP8,
                         kind="ExternalInput").ap()
    wqT = nc.dram_tensor("wqT", [128, JT, CI], FP8, kind="ExternalInput").ap()
    wkT = nc.dram_tensor("wkT", [128, JT, CI], FP8, kind="ExternalInput").ap()
    wvT = nc.dram_tensor("wvT", [128, JT, CI], FP8, kind="ExternalInput").ap()
    wdT = nc.dram_tensor("wdT", [128, IT, CI], FP8, kind="ExternalInput").ap()
    bq = nc.dram_tensor("bq", [CI, 1], F32, kind="ExternalInput").ap()
    bk = nc.dram_tensor("bk", [CI, 1], F32, kind="ExternalInput").ap()
    bv = nc.dram_tensor("bv", [1, CI], BF, kind="ExternalInput").ap()
    bd_f32 = nc.dram_tensor("bd", [CI, 1], F32, kind="ExternalInput").ap()
    alibi = nc.dram_tensor("alibi", [B * HPC, S], F32, kind="ExternalInput").ap()
    residT = nc.dram_tensor("residT", [CI, BS], F32, kind="ExternalInput").ap()
    outT = nc.dram_tensor("outT", [CI, BS], F32, kind="ExternalOutput").ap()

    bounce = nc.dram_tensor("bounce", [N_BLOCKS, HPC, 128, QBLK], FP8,
                            kind="Internal").ap()
    # per-(block, hi) AllGather output: rows = core*128 + d
    gath = nc.dram_tensor("gath", [HPC, N_BLOCKS, N_CORES * 128, QBLK], FP8,
                          kind="Internal", addr_space="Shared").ap()
    if DEBUG_OUTPUTS:
        qT_dbg = nc.dram_tensor("qT_dbg", [128, HPC * BS], BF,
                                kind="ExternalOutput").ap()
        kT_dbg = nc.dram_tensor("kT_dbg", [128, HPC * BS], BF,
                                kind="ExternalOutput").ap()
        v_dbg = nc.dram_tensor("v_dbg", [128, (BS // 128) * CI], BF,
                               kind="ExternalOutput").ap()
        ctxT_dbg = nc.dram_tensor("ctxT_dbg", [H, BS], FP8,
                                  kind="ExternalOutput").ap()

    with tile.TileContext(nc) as tc:
        with (
            tc.tile_pool(name="const", bufs=1) as constp,
            tc.tile_pool(name="qkv", bufs=1) as qkvp,
            tc.tile_pool(name="ctile", bufs=18) as ctp,
        ):
            # ---- phase 0: constants (phase-1 critical ones first) ----
            wq_sb = constp.tile([128, JT, CI], FP8)
            wk_sb = constp.tile([128, JT, CI], FP8)
            wv_sb = constp.tile([128, JT, CI], FP8)
            # wq on the fast gpsimd queue first; wk/wv are emitted inside
            # the chunk loop right after the first hs chunk so the queue
            # order is wq, hs0, wk, wv, hs1, ...
            nc.gpsimd.dma_start(wq_sb[:], wqT[:])
            bq_sb = constp.tile([128, HPC], F32)
            bk_sb = constp.tile([128, HPC], F32)
            for b_sb, b_dr in ((bq_sb, bq), (bk_sb, bk)):
                for hi in range(HPC):
                    nc.scalar.dma_start(b_sb[:, hi:hi + 1],
                                        b_dr[hi * 128:(hi + 1) * 128, :])
            bv_sb = constp.tile([1, CI], BF)
            nc.scalar.dma_start(bv_sb[:], bv[:])
            alibi_sb = constp.tile([128, B * HPC, KT], F32)
            nc.scalar.dma_start(
                alibi_sb[:], alibi.rearrange("r (kt p) -> p r kt", p=128))
            ones_col_bf = constp.tile([128, 1], BF)    # den lhsT (K=128, M=1)
            ones_row_bf = constp.tile([1, 128], BF)    # bias/bcast lhsT
            nc.vector.memset(ones_col_bf[:], 1.0)
            nc.vector.memset(ones_row_bf[:], 1.0)

            # persistent per-core activations
            qT_sb = qkvp.tile([128, HPC, BS], BF)      # [d, hi, ss]
            kT_sb = qkvp.tile([128, HPC, BS], BF)
            v_sb = qkvp.tile([128, BS // 128, CI], BF)  # [ss%128, ss//128, i]

            # ---- phase 1: QKV projections ----
            with (
                tc.tile_pool(name="hsb", bufs=3) as hsp,
                tc.tile_pool(name="p1psum", bufs=4,
                             space=bass.MemorySpace.PSUM) as p1p,
            ):
                for ch in range(N_CHUNKS):
                    s0 = ch * SS_CHUNK
                    hsb = hsp.tile([128, JT, SS_CHUNK], FP8, name="hsb")
                    nc.gpsimd.dma_start(hsb[:], hsT[:, ch])
                    if ch == 0:
                        nc.gpsimd.dma_start(wk_sb[:], wkT[:])
                        nc.gpsimd.dma_start(wv_sb[:], wvT[:])
                    for w_sb, b_col, o_sb, scale in (
                        (wq_sb, bq_sb, qT_sb, INV_NORM / WS),
                        (wk_sb, bk_sb, kT_sb, 1.0 / WS),
                    ):
                        for hi in range(HPC):
                            ps = p1p.tile([128, SS_CHUNK], F32, name="ps_qk")
                            for jp in range(JP):
                                nc.tensor.matmul(
                                    ps[:],
                                    w_sb[:, 2 * jp:2 * jp + 2,
                                         hi * 128:(hi + 1) * 128],
                                    hsb[:, 2 * jp:2 * jp + 2, :],
                                    start=(jp == 0), stop=(jp == JP - 1),
                                    perf_mode=DR)
                            nc.scalar.activation(
                                o_sb[:, hi, s0:s0 + SS_CHUNK], ps[:],
                                mybir.ActivationFunctionType.Identity,
                                bias=b_col[:, hi:hi + 1], scale=scale)
                    for st in range(SS_CHUNK // 128):
                        ps = p1p.tile([128, CI], F32, name="ps_v")
                        # bv host-scaled by WS so the 1/WS below restores it
                        nc.tensor.matmul(ps[:], ones_row_bf[:], bv_sb[:],
                                         start=True, stop=False)
                        for jp in range(JP):
                            nc.tensor.matmul(
                                ps[:],
                                hsb[:, 2 * jp:2 * jp + 2,
                                    st * 128:(st + 1) * 128],
                                wv_sb[:, 2 * jp:2 * jp + 2, :],
                                start=False, stop=(jp == JP - 1),
                                perf_mode=DR)
                        nc.scalar.activation(
                            v_sb[:, ch * 4 + st, :], ps[:],
                            mybir.ActivationFunctionType.Identity,
                            scale=1.0 / WS)

            # late consts (dense phase only) — declared after phase 1 so
            # their DMAs don't delay the first projections
            wd_sb = constp.tile([128, IT, CI], FP8)
            nc.sync.dma_start(wd_sb[:], wdT[:])
            bd_col = constp.tile([128, HPC], F32)
            for ci in range(HPC):
                nc.sync.dma_start(bd_col[:, ci:ci + 1],
                                  bd_f32[ci * 128:(ci + 1) * 128, :])

            # ---- phase 2+3: attention blocks + chunked AllGather ----
            with (
                tc.tile_pool(name="stp", bufs=3,
                             space=bass.MemorySpace.PSUM) as stp,
                tc.tile_pool(name="ptp", bufs=12) as ptp,
                tc.tile_pool(name="accp", bufs=1,
                             space=bass.MemorySpace.PSUM) as accp,
                tc.tile_pool(name="normp", bufs=2) as normp,
            ):
                LAG = 6
                pending_tail = [None]
                ctiles = {}

                def prefetch_ctiles(blk):
                    # DoubleRow pairs: ctile j holds global heads (2j, 2j+1)
                    # = (core j, hi 0/1) stacked along dim1
                    tiles = []
                    for j in range(IT // 2):
                        ctile = ctp.tile([128, 2, QBLK], FP8, name="ctile")
                        for h2 in range(2):
                            nc.gpsimd.dma_start(
                                ctile[:, h2, :],
                                gath[h2, blk, j * 128:(j + 1) * 128, :])
                        tiles.append(ctile)
                    ctiles[blk] = tiles

                def flush_tail():
                    if pending_tail[0] is not None:
                        pending_tail[0]()
                        pending_tail[0] = None

                for blk in range(N_BLOCKS):
                    b, qh = divmod(blk, N_BLOCKS // B)
                    q0 = b * S + qh * QBLK
                    for hi in range(HPC):
                        bh = b * HPC + hi
                        ctx_ps = accp.tile([128, QBLK], F32, name="ctx_ps")
                        acc_sb = normp.tile([128, QBLK], BF, name="acc_sb")
                        pts = []

                        def consume(kt, ctx_ps=ctx_ps, acc_sb=acc_sb,
                                    pts=pts, b=b, hi=hi):
                            pt = pts[kt]
                            for half in range(2):
                                hs_ = slice(half * SS_CHUNK,
                                            (half + 1) * SS_CHUNK)
                                nc.tensor.matmul(
                                    ctx_ps[:, hs_],
                                    v_sb[:, (b * S) // 128 + kt,
                                         hi * 128:(hi + 1) * 128],
                                    pt[:, half, :],
                                    start=(kt == 0), stop=(kt == KT - 1))
                            # denominator partial sums on DVE (off PE):
                            # bf16 pair-sum (2x DVE rate), f32 chain
                            if kt % 2 == 1:
                                pa = pts[kt - 1][:].rearrange(
                                    "p a b -> p (a b)")
                                pb = pt[:].rearrange("p a b -> p (a b)")
                                psum2 = normp.tile([128, QBLK], BF,
                                                   name="psum2")
                                nc.vector.tensor_add(psum2[:], pa, pb)
                                if kt == 1:
                                    nc.vector.tensor_copy(acc_sb[:],
                                                          psum2[:])
                                else:
                                    nc.vector.tensor_add(acc_sb[:],
                                                         acc_sb[:],
                                                         psum2[:])

                        for kt in range(KT):
                            k0 = b * S + kt * 128
                            st_ps = stp.tile([128, 2, SS_CHUNK], F32,
                                             name="st_ps")
                            for half in range(2):
                                nc.tensor.matmul(
                                    st_ps[:, half, :],
                                    kT_sb[:, hi, k0:k0 + 128],
                                    qT_sb[:, hi,
                                          q0 + half * SS_CHUNK:
                                          q0 + (half + 1) * SS_CHUNK],
                                    start=True, stop=True)
                            pt = ptp.tile([128, 2, SS_CHUNK], BF, name="pt")
                            # q pre-scaled by INV_NORM in phase 1; alibi is
                            # a per-partition (key-position) bias
                            nc.scalar.activation(
                                pt[:], st_ps[:],
                                mybir.ActivationFunctionType.Exp,
                                bias=alibi_sb[:, bh, kt:kt + 1])
                            pts.append(pt)
                            # previous group's normalize tail slots in
                            # behind our first few St/exp emissions
                            if kt == 2:
                                flush_tail()
                            if kt >= LAG:
                                consume(kt - LAG)
                        for kt in range(KT - LAG, KT):
                            consume(kt)
                        # cross-partition reduce of acc -> den (borrows an
                        # stp slot; acc chain finishes under the last ctx
                        # matmuls)
                        den_ps = stp.tile([128, 2, SS_CHUNK], F32,
                                          name="st_ps")
                        for half in range(2):
                            nc.tensor.matmul(
                                den_ps[:1, half, :], ones_col_bf[:],
                                acc_sb[:, half * SS_CHUNK:
                                       (half + 1) * SS_CHUNK],
                                start=True, stop=True)
                        den_sb = normp.tile([1, QBLK], BF, name="den_sb")
                        # ACT is idle at the group boundary; DVE is draining
                        # the pair-sum chain — use ACT for this copy.
                        # scale 1/CS so recip(den/CS) = CS/den below.
                        nc.scalar.activation(
                            den_sb[:],
                            den_ps[:1, :, :].rearrange("p a b -> p (a b)"),
                            mybir.ActivationFunctionType.Identity,
                            scale=1.0 / CS)

                        def tail(ctx_ps=ctx_ps, den_sb=den_sb, blk=blk,
                                 hi=hi):
                            denb_ps = stp.tile([128, 2, SS_CHUNK], F32,
                                               name="st_ps")
                            for half in range(2):
                                nc.tensor.matmul(
                                    denb_ps[:, half, :], ones_row_bf[:],
                                    den_sb[:, half * SS_CHUNK:
                                           (half + 1) * SS_CHUNK],
                                    start=True, stop=True)
                            denb_sb = normp.tile([128, QBLK], F32,
                                                 name="denb_sb")
                            nc.vector.reciprocal_approx_fast(
                                denb_sb[:],
                                denb_ps[:].rearrange("p a b -> p (a b)"))
                            ctxn_sb = normp.tile([128, QBLK], FP8,
                                                 name="ctxn_sb")
                            nc.vector.tensor_mul(ctxn_sb[:], ctx_ps[:],
                                                 denb_sb[:])
                            nc.sync.dma_start(bounce[blk, hi], ctxn_sb[:])
                            nc.gpsimd.collective_compute(
                                "AllGather", mybir.AluOpType.bypass,
                                replica_groups=[list(range(N_CORES))],
                                ins=[bounce[blk, hi]],
                                outs=[gath[hi, blk]])
                            # block-0 ctile prefetch slots in on the gpsimd
                            # queue once its gathers are long done
                            if (blk, hi) == (1, 0):
                                prefetch_ctiles(0)

                        pending_tail[0] = tail
                flush_tail()
                for blk in range(1, N_BLOCKS):
                    prefetch_ctiles(blk)

            if DEBUG_OUTPUTS:
                nc.sync.dma_start(qT_dbg[:],
                                  qT_sb[:].rearrange("p a b -> p (a b)"))
                nc.sync.dma_start(kT_dbg[:],
                                  kT_sb[:].rearrange("p a b -> p (a b)"))
                nc.sync.dma_start(v_dbg[:],
                                  v_sb[:].rearrange("p a b -> p (a b)"))
                dbg_r = ctxT_dbg.rearrange("(c x d) s -> c x d s", x=HPC,
                                           d=128)
                for blk in range(N_BLOCKS):
                    b, qh = divmod(blk, N_BLOCKS // B)
                    q0 = b * S + qh * QBLK
                    for hi in range(HPC):
                        nc.sync.dma_start(
                            dbg_r[:, hi, :, q0:q0 + QBLK],
                            gath[hi, blk].rearrange("(c d) s -> c d s",
                                                    d=128))

            # ---- phase 4: output projection (out^T form: Wd stationary,
            # LDWEIGHTS amortized over the moving ctx^T) + bias + residual
            with (
                tc.tile_pool(name="dpsum", bufs=8,
                             space=bass.MemorySpace.PSUM) as dpp,
                tc.tile_pool(name="outp", bufs=4) as outp,
            ):
                NSC = QBLK // SS_CHUNK      # 2 seq chunks per block
                for blk in range(N_BLOCKS):
                    b, qh = divmod(blk, N_BLOCKS // B)
                    q0 = b * S + qh * QBLK
                    dps = [dpp.tile([128, SS_CHUNK], F32, name="dps")
                           for _ in range(HPC * NSC)]
                    for j in range(IT // 2):
                        ctile = ctiles[blk][j]
                        for ct in range(HPC):
                            for sc in range(NSC):
                                nc.tensor.matmul(
                                    dps[ct * NSC + sc][:],
                                    wd_sb[:, 2 * j:2 * j + 2,
                                          ct * 128:(ct + 1) * 128],
                                    ctile[:, :, sc * SS_CHUNK:
                                          (sc + 1) * SS_CHUNK],
                                    start=(j == 0), stop=(j == IT // 2 - 1),
                                    perf_mode=DR)
                    for ct in range(HPC):
                        for sc in range(NSC):
                            c0 = ct * 128
                            s0_ = q0 + sc * SS_CHUNK
                            rtile = outp.tile([128, SS_CHUNK], F32,
                                              name="rtile")
                            nc.sync.dma_start(
                                rtile[:],
                                residT[c0:c0 + 128, s0_:s0_ + SS_CHUNK])
                            # bias is per-partition (output channel) here
                            osb = outp.tile([128, SS_CHUNK], F32,
                                            name="osb")
                            nc.scalar.activation(
                                osb[:], dps[ct * NSC + sc][:],
                                mybir.ActivationFunctionType.Identity,
                                bias=bd_col[:, ct:ct + 1],
                                scale=1.0 / (WS * CS))
                            osb2 = outp.tile([128, SS_CHUNK], F32,
                                             name="osb2")
                            nc.vector.tensor_add(osb2[:], osb[:], rtile[:])
                            nc.sync.dma_start(
                                outT[c0:c0 + 128, s0_:s0_ + SS_CHUNK],
                                osb2[:])

    nc.compile()
    return nc


_NC = None


def _get_nc():
    global _NC
    if _NC is None:
        _NC = _build()
    return _NC


def _pack_w(W, sl):
    # [H, CI] transposed slice -> SBUF layout [128, JT, CI], contiguous.
    # Pre-scaled by WS so fp8 e4m3 normals cover the ~1/sqrt(H) magnitudes.
    wT = np.asarray(W, np.float32)[sl].T * WS       # [H, CI]
    return np.ascontiguousarray(
        wT.reshape(JT, 128, CI).transpose(1, 0, 2)).astype(F8NP)


def _prep_in_maps(hidden_states, residual, alibi, Wq, bq, Wk, bk, Wv, bv,
                  Wd, bd):
    hs = np.ascontiguousarray(np.asarray(hidden_states, np.float32)
                              .reshape(BS, H))
    # SBUF chunk layout [128, ch, jt, s]: element = hs[ch*512+s, jt*128+p]
    hs_pack = np.ascontiguousarray(
        hs.reshape(N_CHUNKS, SS_CHUNK, JT, 128).transpose(3, 0, 2, 1)
    ).astype(F8NP)
    resid = np.asarray(residual, np.float32).reshape(BS, H)
    alibi_r = np.asarray(alibi, np.float32).reshape(B, NH, S)
    in_maps = []
    for c in range(N_CORES):
        sl = slice(c * CI, (c + 1) * CI)
        # alibi rows ordered (b, hi) to match kernel indexing bh = b*HPC+hi
        al = np.ascontiguousarray(
            alibi_r[:, c * HPC:(c + 1) * HPC, :].reshape(B * HPC, S))
        in_maps.append({
            "hsT": hs_pack,
            "wqT": _pack_w(Wq, sl),
            "wkT": _pack_w(Wk, sl),
            "wvT": _pack_w(Wv, sl),
            "wdT": _pack_w(Wd, sl),
            "bq": np.asarray(bq, np.float32)[sl].reshape(CI, 1),
            "bk": np.asarray(bk, np.float32)[sl].reshape(CI, 1),
            "bv": (np.asarray(bv, np.float32)[sl] * WS).reshape(1, CI)
                  .astype(BF16),
            "bd": np.asarray(bd, np.float32)[sl].reshape(CI, 1),
            "alibi": al,
            "residT": np.ascontiguousarray(resid[:, sl].T),
        })
    return in_maps


def run(trace=False, trace_cores=None, stitch_traces=False, **inputs):
    nc = _get_nc()
    in_maps = _prep_in_maps(**inputs)
    res = bass_utils.run_bass_kernel_spmd(
        nc, in_maps, core_ids=list(range(N_CORES)), trace=trace,
        trace_cores=trace_cores, stitch_traces=stitch_traces)
    full = np.empty((BS, H), np.float32)
    for c in range(N_CORES):
        full[:, c * CI:(c + 1) * CI] = res.results[c]["outT"].T
    return full.reshape(B, S, H), res


def kernel(**inputs):
    out, _ = run(trace=False, **inputs)
    return out



# revision 67
# speedup vs baseline: 1.4211x; 1.0090x over previous
"""Bloom attention (separated QKV) — 8-core TRN2 Bass kernel.

Distribution: tensor-parallel over heads (2 heads/core). Each core:
  1. QKV projections for its 256-row slice of Wq/Wk/Wv via fp8e4
     DoubleRow matmuls (2 k-tiles per pass, ~2x bf16 rate): hs and
     weights are host-cast to fp8 (weights pre-scaled x64 so e4m3
     normals cover them; undone in the PSUM->SBUF activation). q^T/k^T
     land in [d,s] bf16, v in [s,d] bf16.
  2. Attention with transposed scores St[k,q] = k @ q^T (bf16, K=128 so
     no DoubleRow), exp via ScalarE (alibi as per-partition bias),
     denominator via one [128,128] ones(1/32)-matmul per half (reduce +
     broadcast in one shot), ctx^T = v^T @ P in PSUM, normalized by
     reciprocal and cast to fp8 (x32 so e4m3 normals cover ctx).
  3. One AllGather per 1024-query block (both heads, fp8 payload).
  4. Dense projection from the gathered fp8 ctx^T via DoubleRow pairs +
     bias + residual — interleaved with attention at block granularity
     (dense(0) before the last attention block, rest after) so the last
     AllGather's latency+skew hides under dense work.
Startup DMAs are spread over the sync/scalar/gpsimd queues in need-order
(per-queue DMA ~125GB/s; a gated DMA trigger blocks its queue head-of-
line, so bulk transfers stay off the ACT queue once exps start).
Host side: transpose/slice/cast weights + hs (layout prep only),
concatenate the 8 output column-slices.
"""
import numpy as np
import ml_dtypes

import concourse.bass as bass
import concourse.bacc as bacc
import concourse.mybir as mybir
import concourse.tile as tile
import concourse.bass_utils as bass_utils

BF16 = ml_dtypes.bfloat16
F8NP = ml_dtypes.float8_e4m3
N_CORES = 8
B, S, H = 2, 2048, 2048
NH, HD = 16, 128
HPC = NH // N_CORES          # heads per core
CI = HPC * HD                # per-core slice of H (256)
BS = B * S                   # 4096
INV_NORM = 1.0 / float(np.sqrt(HD))
WS = 64.0                    # fp8 weight pre-scale (host), undone in ACT
CS = 32.0                    # ctx pre-scale before fp8 cast, undone in ACT

JT = H // 128                # 16 contraction tiles for projections
JP = JT // 2                 # 8 DoubleRow k-tile pairs
SS_CHUNK = 512               # seq chunk for projections
N_CHUNKS = BS // SS_CHUNK    # 8
KT = S // 128                # 16 key tiles per batch
IT = H // 128                # 16 contraction tiles for dense
QBLK = 1024                  # attention/AG/dense block along seq
N_BLOCKS = BS // QBLK        # 4

F32 = mybir.dt.float32
BF = mybir.dt.bfloat16
FP8 = mybir.dt.float8e4
DR = mybir.MatmulPerfMode.DoubleRow

DEBUG_OUTPUTS = False


def _build():
    nc = bacc.Bacc("TRN2", target_bir_lowering=False, debug=False,
                   num_devices=N_CORES)

    # hsT/weights are host-packed to the exact SBUF layouts so every DMA
    # is contiguous per partition (strided weight loads measured ~5x slower)
    hsT = nc.dram_tensor("hsT", [128, N_CHUNKS, JT, SS_CHUNK], FP8,
                         kind="ExternalInput").ap()
    wqT = nc.dram_tensor("wqT", [128, JT, CI], FP8, kind="ExternalInput").ap()
    wkT = nc.dram_tensor("wkT", [128, JT, CI], FP8, kind="ExternalInput").ap()
    wvT = nc.dram_tensor("wvT", [128, JT, CI], FP8, kind="ExternalInput").ap()
    wdT = nc.dram_tensor("wdT", [128, IT, CI], FP8, kind="ExternalInput").ap()
    bq = nc.dram_tensor("bq", [CI, 1], F32, kind="ExternalInput").ap()
    bk = nc.dram_tensor("bk", [CI, 1], F32, kind="ExternalInput").ap()
    bv = nc.dram_tensor("bv", [1, CI], BF, kind="ExternalInput").ap()
    bd_f32 = nc.dram_tensor("bd", [CI, 1], F32, kind="ExternalInput").ap()
    alibi = nc.dram_tensor("alibi", [B * HPC, S], F32, kind="ExternalInput").ap()
    residT = nc.dram_tensor("residT", [CI, BS], F32, kind="ExternalInput").ap()
    outT = nc.dram_tensor("outT", [CI, BS], F32, kind="ExternalOutput").ap()

    bounce = nc.dram_tensor("bounce", [N_BLOCKS, HPC, 128, QBLK], FP8,
                            kind="Internal").ap()
    # per-block AllGather output (both heads per core in one collective —
    # AG cost is latency-dominated, so fewer, bigger AGs win)
    gath = nc.dram_tensor("gath", [N_BLOCKS, N_CORES, HPC, 128, QBLK], FP8,
                          kind="Internal", addr_space="Shared").ap()
    # block 3 gathers per head so AG(3,0) hides under A(3,1)'s compute
    gath3 = nc.dram_tensor("gath3", [HPC, N_CORES, 128, QBLK], FP8,
                           kind="Internal", addr_space="Shared").ap()
    if DEBUG_OUTPUTS:
        qT_dbg = nc.dram_tensor("qT_dbg", [128, HPC * BS], BF,
                                kind="ExternalOutput").ap()
        kT_dbg = nc.dram_tensor("kT_dbg", [128, HPC * BS], BF,
                                kind="ExternalOutput").ap()
        v_dbg = nc.dram_tensor("v_dbg", [128, (BS // 128) * CI], BF,
                               kind="ExternalOutput").ap()
        ctxT_dbg = nc.dram_tensor("ctxT_dbg", [H, BS], FP8,
                                  kind="ExternalOutput").ap()

    with tile.TileContext(nc) as tc:
        with (
            tc.tile_pool(name="const", bufs=1) as constp,
            tc.tile_pool(name="qkv", bufs=1) as qkvp,
            tc.tile_pool(name="ctile", bufs=3) as ctp,
        ):
            # ---- phase 0: constants (phase-1 critical ones first) ----
            wq_sb = constp.tile([128, JT, CI], FP8)
            wk_sb = constp.tile([128, JT, CI], FP8)
            wv_sb = constp.tile([128, JT, CI], FP8)
            # startup-critical DMAs spread over all three queues in
            # need-order (per-queue DMA rate is only ~125GB/s, so the naive
            # single-queue order stalls phase 1 repeatedly):
            #   gpsimd: wq(4us) wk(8) wv(12) hs3..    | first q matmul
            #   sync:   hs0_h1(4) hs1(12) hs2(20)     | needs wq_h1+hs0_h1
            #   scalar: hs0_h2(4) only — bulk DMAs on | at ~13us
            #   the ACT queue stall its ACTIVATEs (measured 16us trigger)
            for h in range(2):
                sl = slice(h * JT // 2, (h + 1) * JT // 2)
                nc.gpsimd.dma_start(wq_sb[:, sl], wqT[:, sl])
            bq_sb = constp.tile([128, HPC], F32)
            bk_sb = constp.tile([128, HPC], F32)
            for b_sb, b_dr in ((bq_sb, bq), (bk_sb, bk)):
                for hi in range(HPC):
                    nc.scalar.dma_start(b_sb[:, hi:hi + 1],
                                        b_dr[hi * 128:(hi + 1) * 128, :])
            bv_sb = constp.tile([1, CI], BF)
            nc.scalar.dma_start(bv_sb[:], bv[:])
            alibi_sb = constp.tile([128, B * HPC, KT], F32)
            nc.scalar.dma_start(
                alibi_sb[:], alibi.rearrange("r (kt p) -> p r kt", p=128))
            # den reduce+broadcast lhsT: [128,128] of 1/CS — one matmul per
            # half gives denb[m,q] = den[q]/CS on every output partition
            ones128 = constp.tile([128, 128], BF)
            ones_row_bf = constp.tile([1, 128], BF)    # v-bias lhsT
            nc.vector.memset(ones128[:], 1.0 / CS)
            nc.vector.memset(ones_row_bf[:], 1.0)

            # persistent per-core activations
            qT_sb = qkvp.tile([128, HPC, BS], BF)      # [d, hi, ss]
            kT_sb = qkvp.tile([128, HPC, BS], BF)
            v_sb = qkvp.tile([128, BS // 128, CI], BF)  # [ss%128, ss//128, i]

            # ---- phase 1: QKV projections ----
            with (
                tc.tile_pool(name="hsb", bufs=4) as hsp,
                tc.tile_pool(name="p1psum", bufs=4,
                             space=bass.MemorySpace.PSUM) as p1p,
            ):
                # chunk 0 halves ride the idle sync+scalar queues so its
                # first matmul only waits for wq_h1 + hs0_h1; later chunks
                # are prefetched one iteration ahead on gpsimd, giving the
                # queue order wq, hs1, wk, wv, hs2, ... (wk is needed ~4us
                # after the first q matmuls start, hs1 ~9us later)
                for ch in range(N_CHUNKS):
                    s0 = ch * SS_CHUNK
                    hsb = hsp.tile([128, JT, SS_CHUNK], FP8, name="hsb")
                    if ch == 0:
                        # need-order, halved, spread over queues; scalar
                        # gets only one early piece (ACT needs the queue
                        # from ~18us). Triggers run ahead of the PE, so
                        # later chunks prefetch via queue runahead.
                        nc.sync.dma_start(hsb[:, :JT // 2],
                                          hsT[:, 0, :JT // 2])
                        nc.scalar.dma_start(hsb[:, JT // 2:],
                                            hsT[:, 0, JT // 2:])
                        for h in range(2):
                            sl = slice(h * JT // 2, (h + 1) * JT // 2)
                            nc.gpsimd.dma_start(wk_sb[:, sl], wkT[:, sl])
                        for h in range(2):
                            sl = slice(h * JT // 2, (h + 1) * JT // 2)
                            nc.gpsimd.dma_start(wv_sb[:, sl], wvT[:, sl])
                    elif ch == 1:
                        for h in range(2):
                            sl = slice(h * JT // 2, (h + 1) * JT // 2)
                            nc.sync.dma_start(hsb[:, sl], hsT[:, 1, sl])
                    else:
                        # even chunks on gpsimd (free after wv), odd on sync
                        q = nc.gpsimd if ch % 2 == 0 else nc.sync
                        q.dma_start(hsb[:], hsT[:, ch])
                    for w_sb, b_col, o_sb, scale in (
                        (wq_sb, bq_sb, qT_sb, INV_NORM / WS),
                        (wk_sb, bk_sb, kT_sb, 1.0 / WS),
                    ):
                        for hi in range(HPC):
                            ps = p1p.tile([128, SS_CHUNK], F32, name="ps_qk")
                            for jp in range(JP):
                                nc.tensor.matmul(
                                    ps[:],
                                    w_sb[:, 2 * jp:2 * jp + 2,
                                         hi * 128:(hi + 1) * 128],
                                    hsb[:, 2 * jp:2 * jp + 2, :],
                                    start=(jp == 0), stop=(jp == JP - 1),
                                    perf_mode=DR)
                            nc.scalar.activation(
                                o_sb[:, hi, s0:s0 + SS_CHUNK], ps[:],
                                mybir.ActivationFunctionType.Identity,
                                bias=b_col[:, hi:hi + 1], scale=scale)
                    for st in range(SS_CHUNK // 128):
                        ps = p1p.tile([128, CI], F32, name="ps_v")
                        # bv host-scaled by WS so the 1/WS below restores it
                        nc.tensor.matmul(ps[:], ones_row_bf[:], bv_sb[:],
                                         start=True, stop=False)
                        for jp in range(JP):
                            nc.tensor.matmul(
                                ps[:],
                                hsb[:, 2 * jp:2 * jp + 2,
                                    st * 128:(st + 1) * 128],
                                wv_sb[:, 2 * jp:2 * jp + 2, :],
                                start=False, stop=(jp == JP - 1),
                                perf_mode=DR)
                        nc.scalar.activation(
                            v_sb[:, ch * 4 + st, :], ps[:],
                            mybir.ActivationFunctionType.Identity,
                            scale=1.0 / WS)

            # late consts (dense phase only) — declared after phase 1 so
            # their DMAs don't delay the first projections
            wd_sb = constp.tile([128, IT, CI], FP8)
            nc.sync.dma_start(wd_sb[:], wdT[:])
            bd_col = constp.tile([128, HPC], F32)
            for ci in range(HPC):
                nc.sync.dma_start(bd_col[:, ci:ci + 1],
                                  bd_f32[ci * 128:(ci + 1) * 128, :])

            # ---- phase 2+3+4: attention + chunked AllGather, with the
            # dense output projection interleaved at block granularity so
            # the last blocks' AllGather latency hides under dense work
            with (
                tc.tile_pool(name="stp", bufs=3,
                             space=bass.MemorySpace.PSUM) as stp,
                tc.tile_pool(name="ptp", bufs=12) as ptp,
                tc.tile_pool(name="accp", bufs=1,
                             space=bass.MemorySpace.PSUM) as accp,
                tc.tile_pool(name="normp", bufs=2) as normp,
                tc.tile_pool(name="outp", bufs=4) as outp,
            ):
                LAG = 6
                NSC = QBLK // SS_CHUNK      # 2 seq chunks per block
                pending_tail = [None]
                ctiles = {}

                def prefetch_ctiles(blk):
                    # DoubleRow pairs: dim1 j = pair of global heads
                    # (2j, 2j+1) = (core j, hi 0/1). Two batched DMAs per
                    # block (a trigger costs ~630ns of queue issue time, so
                    # 16 singles would put ~10us on the critical path).
                    # Queue placement matters too: a gated DMA trigger
                    # blocks its whole queue head-of-line, so prefetches
                    # are emitted only where their AG is already done (or
                    # nothing urgent sits behind them).
                    ctile = ctp.tile([128, IT // 2, 2, QBLK], FP8,
                                     name="ctile")
                    if blk == 3:
                        # per-head sources; slot j = hi*4 + p holds cores
                        # (2p, 2p+1) of head-parity hi, i.e. the same-
                        # parity DoubleRow pair (4p+hi, 4p+2+hi). hi=0
                        # pieces are gated on AG(3,0) (done early); only
                        # hi=1 pieces wait on AG(3,1). Alternate queues.
                        for hi in range(HPC):
                            src3 = gath3[hi].rearrange("c d q -> d c q")
                            for p in range(4):
                                q = nc.gpsimd if p % 2 == 0 else nc.sync
                                q.dma_start(
                                    ctile[:, hi * 4 + p],
                                    src3[:, 2 * p:2 * p + 2])
                    else:
                        src = gath[blk].rearrange("c h d q -> d c h q")
                        for p4 in range(4):
                            nc.gpsimd.dma_start(
                                ctile[:, 2 * p4:2 * p4 + 2],
                                src[:, 2 * p4:2 * p4 + 2])
                    ctiles[blk] = ctile

                def flush_tail():
                    if pending_tail[0] is not None:
                        pending_tail[0]()
                        pending_tail[0] = None

                def attn_group(blk, hi):
                    b, qh = divmod(blk, N_BLOCKS // B)
                    q0 = b * S + qh * QBLK
                    bh = b * HPC + hi
                    ctx_ps = accp.tile([128, QBLK], F32, name="ctx_ps")
                    acc_sb = normp.tile([128, QBLK], BF, name="acc_sb")
                    pts = []

                    def consume(kt):
                        pt = pts[kt]
                        for half in range(2):
                            hs_ = slice(half * SS_CHUNK,
                                        (half + 1) * SS_CHUNK)
                            nc.tensor.matmul(
                                ctx_ps[:, hs_],
                                v_sb[:, (b * S) // 128 + kt,
                                     hi * 128:(hi + 1) * 128],
                                pt[:, half, :],
                                start=(kt == 0), stop=(kt == KT - 1))
                        # denominator partial sums on DVE (off PE):
                        # bf16 pair-sum (2x DVE rate)
                        if kt % 2 == 1:
                            pa = pts[kt - 1][:].rearrange("p a b -> p (a b)")
                            pb = pt[:].rearrange("p a b -> p (a b)")
                            psum2 = normp.tile([128, QBLK], BF,
                                               name="psum2")
                            nc.vector.tensor_add(psum2[:], pa, pb)
                            if kt == 1:
                                nc.vector.tensor_copy(acc_sb[:], psum2[:])
                            else:
                                nc.vector.tensor_add(acc_sb[:], acc_sb[:],
                                                     psum2[:])

                    for kt in range(KT):
                        k0 = b * S + kt * 128
                        st_ps = stp.tile([128, 2, SS_CHUNK], F32,
                                         name="st_ps")
                        for half in range(2):
                            nc.tensor.matmul(
                                st_ps[:, half, :],
                                kT_sb[:, hi, k0:k0 + 128],
                                qT_sb[:, hi,
                                      q0 + half * SS_CHUNK:
                                      q0 + (half + 1) * SS_CHUNK],
                                start=True, stop=True)
                        pt = ptp.tile([128, 2, SS_CHUNK], BF, name="pt")
                        # q pre-scaled by INV_NORM in phase 1; alibi is
                        # a per-partition (key-position) bias
                        nc.scalar.activation(
                            pt[:], st_ps[:],
                            mybir.ActivationFunctionType.Exp,
                            bias=alibi_sb[:, bh, kt:kt + 1])
                        pts.append(pt)
                        # previous group's normalize tail slots in
                        # behind our first few St/exp emissions
                        if kt == 2:
                            flush_tail()
                        if kt >= LAG:
                            consume(kt - LAG)
                    for kt in range(KT - LAG, KT):
                        consume(kt)
                    # den reduce+broadcast in one matmul per half:
                    # denb[m,q] = den[q]/CS for all m (ones128 = 1/CS)
                    den_ps = stp.tile([128, 2, SS_CHUNK], F32,
                                      name="st_ps")
                    for half in range(2):
                        nc.tensor.matmul(
                            den_ps[:, half, :], ones128[:],
                            acc_sb[:, half * SS_CHUNK:
                                   (half + 1) * SS_CHUNK],
                            start=True, stop=True)

                    def tail():
                        denb_sb = normp.tile([128, QBLK], F32,
                                             name="denb_sb")
                        nc.vector.reciprocal_approx_fast(
                            denb_sb[:],
                            den_ps[:].rearrange("p a b -> p (a b)"))
                        ctxn_sb = normp.tile([128, QBLK], FP8,
                                             name="ctxn_sb")
                        nc.vector.tensor_mul(ctxn_sb[:], ctx_ps[:],
                                             denb_sb[:])
                        nc.sync.dma_start(bounce[blk, hi], ctxn_sb[:])
                        if blk == 3:
                            # per-head AGs for the last block: AG(3,0)'s
                            # mesh runs during A(3,1)'s compute, so only
                            # hi=1's gather remains on the critical tail
                            nc.gpsimd.collective_compute(
                                "AllGather", mybir.AluOpType.bypass,
                                replica_groups=[list(range(N_CORES))],
                                ins=[bounce[blk, hi]],
                                outs=[gath3[hi]])
                        elif hi == 1:
                            # one AG per block, both heads' bounce slices
                            nc.gpsimd.collective_compute(
                                "AllGather", mybir.AluOpType.bypass,
                                replica_groups=[list(range(N_CORES))],
                                ins=[bounce[blk]],
                                outs=[gath[blk]])
                        if (blk, hi) == (1, 0):
                            prefetch_ctiles(0)
                        elif (blk, hi) == (2, 0):
                            prefetch_ctiles(1)
                        elif (blk, hi) == (3, 1):
                            # after AG(3)'s trigger so it can't delay it
                            prefetch_ctiles(2)

                    pending_tail[0] = tail

                def dense_block(blk):
                    flush_tail()
                    b, qh = divmod(blk, N_BLOCKS // B)
                    q0 = b * S + qh * QBLK
                    if blk == 3:
                        # hi-outer with both ct accumulators alive: all 16
                        # hi=0 matmuls (gated only on the early AG(3,0))
                        # run during AG(3,1)'s mesh + transfer
                        dpairs = [stp.tile([128, NSC, SS_CHUNK], F32,
                                           name="st_ps")
                                  for _ in range(HPC)]
                        for hi_ in range(2):
                            for ct in range(HPC):
                                for p_ in range(4):
                                    g0 = 4 * p_ + hi_
                                    wsl = wd_sb[:, g0:g0 + 3:2,
                                                ct * 128:(ct + 1) * 128]
                                    for sc in range(NSC):
                                        nc.tensor.matmul(
                                            dpairs[ct][:, sc, :], wsl,
                                            ctiles[3][:, hi_ * 4 + p_, :,
                                                      sc * SS_CHUNK:
                                                      (sc + 1) * SS_CHUNK],
                                            start=(hi_ == 0 and p_ == 0),
                                            stop=(hi_ == 1 and p_ == 3),
                                            perf_mode=DR)
                        for ct in range(HPC):
                            for sc in range(NSC):
                                c0 = ct * 128
                                s0_ = q0 + sc * SS_CHUNK
                                rtile = outp.tile([128, SS_CHUNK], F32,
                                                  name="rtile")
                                nc.sync.dma_start(
                                    rtile[:],
                                    residT[c0:c0 + 128,
                                           s0_:s0_ + SS_CHUNK])
                                osb = outp.tile([128, SS_CHUNK], F32,
                                                name="osb")
                                nc.scalar.activation(
                                    osb[:], dpairs[ct][:, sc, :],
                                    mybir.ActivationFunctionType.Identity,
                                    bias=bd_col[:, ct:ct + 1],
                                    scale=1.0 / (WS * CS))
                                osb2 = outp.tile([128, SS_CHUNK], F32,
                                                 name="osb2")
                                nc.vector.tensor_add(osb2[:], osb[:],
                                                     rtile[:])
                                nc.sync.dma_start(
                                    outT[c0:c0 + 128, s0_:s0_ + SS_CHUNK],
                                    osb2[:])
                        return
                    for ct in range(HPC):
                        # accumulators borrow an stp ring slot (dense runs
                        # at attention-group boundaries, so at most one
                        # slot is held while the neighbors pipeline)
                        dpair = stp.tile([128, NSC, SS_CHUNK], F32,
                                         name="st_ps")
                        for j in range(IT // 2):
                            if blk == 3:
                                # slot j = hi*4+p pairs same-parity heads
                                # (4p+hi, 4p+2+hi) -> strided wd slice
                                hi_, p_ = divmod(j, 4)
                                g0 = 4 * p_ + hi_
                                wsl = wd_sb[:, g0:g0 + 3:2,
                                            ct * 128:(ct + 1) * 128]
                            else:
                                wsl = wd_sb[:, 2 * j:2 * j + 2,
                                            ct * 128:(ct + 1) * 128]
                            for sc in range(NSC):
                                nc.tensor.matmul(
                                    dpair[:, sc, :], wsl,
                                    ctiles[blk][:, j, :, sc * SS_CHUNK:
                                                (sc + 1) * SS_CHUNK],
                                    start=(j == 0), stop=(j == IT // 2 - 1),
                                    perf_mode=DR)
                        for sc in range(NSC):
                            c0 = ct * 128
                            s0_ = q0 + sc * SS_CHUNK
                            rtile = outp.tile([128, SS_CHUNK], F32,
                                              name="rtile")
                            nc.sync.dma_start(
                                rtile[:],
                                residT[c0:c0 + 128, s0_:s0_ + SS_CHUNK])
                            # bias is per-partition (output channel) here
                            osb = outp.tile([128, SS_CHUNK], F32,
                                            name="osb")
                            nc.scalar.activation(
                                osb[:], dpair[:, sc, :],
                                mybir.ActivationFunctionType.Identity,
                                bias=bd_col[:, ct:ct + 1],
                                scale=1.0 / (WS * CS))
                            osb2 = outp.tile([128, SS_CHUNK], F32,
                                             name="osb2")
                            nc.vector.tensor_add(osb2[:], osb[:], rtile[:])
                            nc.sync.dma_start(
                                outT[c0:c0 + 128, s0_:s0_ + SS_CHUNK],
                                osb2[:])

                # schedule: dense(0) before the last attention block, the
                # rest after — AG(3,*) latency hides under dense(1..2)
                for blk in range(3):
                    for hi in range(HPC):
                        attn_group(blk, hi)
                dense_block(0)
                for hi in range(HPC):
                    attn_group(3, hi)
                flush_tail()
                dense_block(1)
                dense_block(2)
                # emitted only now: gpsimd DMA-completion semaphores count
                # in queue order, so anything emitted after these waits for
                # AG(3,*) too — only dense(3) may.
                prefetch_ctiles(3)
                # The AG(3) skew wait (~17-35us PE-idle) makes HAM
                # re-throttle, and dense(3) then runs at the 1.2GHz p-state
                # (427ns/mm measured). ~8us of dependency-free filler
                # matmuls into a scratch PSUM slot extend the busy window
                # into the wait; they can only cost time if the AG ever
                # finished within 8us of dense(2), which it never does.
                warm_ps = stp.tile([128, 2, SS_CHUNK], F32, name="st_ps")
                for w in range(30):
                    nc.tensor.matmul(
                        warm_ps[:, w % 2, :], ones128[:],
                        kT_sb[:, 0, :SS_CHUNK], start=True, stop=True)
                dense_block(3)

            if DEBUG_OUTPUTS:
                nc.sync.dma_start(qT_dbg[:],
                                  qT_sb[:].rearrange("p a b -> p (a b)"))
                nc.sync.dma_start(kT_dbg[:],
                                  kT_sb[:].rearrange("p a b -> p (a b)"))
                nc.sync.dma_start(v_dbg[:],
                                  v_sb[:].rearrange("p a b -> p (a b)"))
                dbg_r = ctxT_dbg.rearrange("(c x d) s -> c x d s", x=HPC,
                                           d=128)
                for blk in range(N_BLOCKS):
                    b, qh = divmod(blk, N_BLOCKS // B)
                    q0 = b * S + qh * QBLK
                    for hi in range(HPC):
                        nc.sync.dma_start(
                            dbg_r[:, hi, :, q0:q0 + QBLK],
                            gath[blk, :, hi])

    nc.compile()
    return nc


_NC = None


def _get_nc():
    global _NC
    if _NC is None:
        _NC = _build()
    return _NC


def _pack_w(W, sl):
    # [H, CI] transposed slice -> SBUF layout [128, JT, CI], contiguous.
    # Pre-scaled by WS so fp8 e4m3 normals cover the ~1/sqrt(H) magnitudes.
    wT = np.asarray(W, np.float32)[sl].T * WS       # [H, CI]
    return np.ascontiguousarray(
        wT.reshape(JT, 128, CI).transpose(1, 0, 2)).astype(F8NP)


def _prep_in_maps(hidden_states, residual, alibi, Wq, bq, Wk, bk, Wv, bv,
                  Wd, bd):
    hs = np.ascontiguousarray(np.asarray(hidden_states, np.float32)
                              .reshape(BS, H))
    # SBUF chunk layout [128, ch, jt, s]: element = hs[ch*512+s, jt*128+p]
    hs_pack = np.ascontiguousarray(
        hs.reshape(N_CHUNKS, SS_CHUNK, JT, 128).transpose(3, 0, 2, 1)
    ).astype(F8NP)
    resid = np.asarray(residual, np.float32).reshape(BS, H)
    alibi_r = np.asarray(alibi, np.float32).reshape(B, NH, S)
    in_maps = []
    for c in range(N_CORES):
        sl = slice(c * CI, (c + 1) * CI)
        # alibi rows ordered (b, hi) to match kernel indexing bh = b*HPC+hi
        al = np.ascontiguousarray(
            alibi_r[:, c * HPC:(c + 1) * HPC, :].reshape(B * HPC, S))
        in_maps.append({
            "hsT": hs_pack,
            "wqT": _pack_w(Wq, sl),
            "wkT": _pack_w(Wk, sl),
            "wvT": _pack_w(Wv, sl),
            "wdT": _pack_w(Wd, sl),
            "bq": np.asarray(bq, np.float32)[sl].reshape(CI, 1),
            "bk": np.asarray(bk, np.float32)[sl].reshape(CI, 1),
            "bv": (np.asarray(bv, np.float32)[sl] * WS).reshape(1, CI)
                  .astype(BF16),
            "bd": np.asarray(bd, np.float32)[sl].reshape(CI, 1),
            "alibi": al,
            "residT": np.ascontiguousarray(resid[:, sl].T),
        })
    return in_maps


def run(trace=False, trace_cores=None, stitch_traces=False, **inputs):
    nc = _get_nc()
    in_maps = _prep_in_maps(**inputs)
    res = bass_utils.run_bass_kernel_spmd(
        nc, in_maps, core_ids=list(range(N_CORES)), trace=trace,
        trace_cores=trace_cores, stitch_traces=stitch_traces)
    full = np.empty((BS, H), np.float32)
    for c in range(N_CORES):
        full[:, c * CI:(c + 1) * CI] = res.results[c]["outT"].T
    return full.reshape(B, S, H), res


def kernel(**inputs):
    out, _ = run(trace=False, **inputs)
    return out

